# revision 25
# baseline (speedup 1.0000x reference)
"""Ring-attention (context-parallel) kernel for 8 TRN2 NeuronCores.

Problem: x_q [1,2048,2048], x_kv [1,8192,2048], GQA attention (16 q heads,
4 kv heads, D=128) where q occupies global positions 6144..8191 of the
8192-long key sequence (causal on the last 2048 block, full attention on
the first 6144 keys), followed by an output projection.

Strategy (sequence/context parallel):
  - q rows split into 16 strips of 128; core c owns strips {c, 15-c}
    (folded pairing balances the causal wedge).
  - x_kv sequence-sharded 8 x 1024 rows; each core projects its local
    K/V shard, then one AllGather PER HEAD GROUP shares the full K^T / V.
    The V projection runs first and each AG launches right after its
    group's K^T finishes, so AG0 completes long before attention needs it.
  - Attention q-columns are ordered (strip, head, q128) so every S matmul
    is a single N=512 pass per strip sharing one K-chunk weight load, and
    fully-causal-masked strip halves are skipped (kc>=56 computes only the
    high strip).
  - Softmax denominator: bf16 chunk accumulation on DVE, then one
    gpsimd.partition_all_reduce (sum over partitions, broadcast) per head
    group -- no TensorE ones-matmuls, no extra PSUM banks.
  - Output projection for group g is emitted after group g+1's attention
    so the normalization chain never stalls the in-order PE queue.
"""

import numpy as np
import ml_dtypes

import concourse.bass as bass
import concourse.mybir as mybir
import concourse.tile as tile
from concourse import bacc, bass_isa, bass_utils

BF16 = ml_dtypes.bfloat16
F32 = mybir.dt.float32
BF = mybir.dt.bfloat16

N_CORES = 8
H = 16          # query heads
HKV = 4         # kv heads
D = 128         # head dim
HID = H * D     # 2048
SL = 2048       # q rows (global)
SKV = 8192      # kv rows (global)
QS = 256        # q rows per core (2 strips of 128)
LKV = SKV // N_CORES   # 1024 local kv rows
HC = HID // 128        # 16 hid chunks
KC = SKV // 128        # 64 key chunks
RANK_OFF = SKV - SL    # 6144: global position of q row 0
BND = RANK_OFF // 128  # 48: first key chunk needing a causal mask
SCALE = 1.0 / float(np.sqrt(D))

_CACHE = {}


def _build():
    nc = bacc.Bacc("TRN2", target_bir_lowering=False, debug=False,
                   num_devices=N_CORES)

    xqT = nc.dram_tensor("xqT", [HID, QS], BF, kind="ExternalInput")
    xkvT = nc.dram_tensor("xkvT", [HID, LKV], BF, kind="ExternalInput")
    wqP = nc.dram_tensor("wqP", [H * HID, D], BF, kind="ExternalInput")
    wkT = nc.dram_tensor("wkT", [HID, HKV * D], BF, kind="ExternalInput")
    wvT = nc.dram_tensor("wvT", [HID, HKV * D], BF, kind="ExternalInput")
    woT = nc.dram_tensor("woT", [HID, HID], BF, kind="ExternalInput")
    # boundary causal masks, (strip,head,q) col order: j<8 -> strip0 mask,
    # j>=8 -> strip1 mask, each [128 keys, 4 heads x 128 q]
    maskD = nc.dram_tensor("mask", [16 * 128, 512], BF, kind="ExternalInput")
    outT = nc.dram_tensor("outT", [HID, QS], F32, kind="ExternalOutput")

    with tile.TileContext(nc) as tc:
        _body(nc, tc, xqT, xkvT, wqP, wkT, wvT, woT, maskD, outT)
    nc.compile()
    return nc


def _body(nc, tc, xqT, xkvT, wqP, wkT, wvT, woT, maskD, outT):
    from contextlib import ExitStack
    ctx = ExitStack()
    with ctx:
        persist = ctx.enter_context(tc.tile_pool(name="persist", bufs=1))
        dram = ctx.enter_context(tc.tile_pool(name="dram", bufs=1, space="DRAM"))

        # resident tiles
        qt_sb = persist.tile([128, 2, H, 128], BF)    # Q^T [D, strip, head, q]
        ao_sb = persist.tile([128, HKV, 4, QS], BF)   # normalized O^T per g
        out_acc = persist.tile([128, HC, QS], F32)

        # per-g AllGather bounce: bytes [0:128*LKV) = K^T_g [D, LKV],
        # [128*LKV:256*LKV) = V_g [LKV, D] row-major
        bnc = [dram.tile([256 * LKV], BF, name=f"bnc{g}", uniquify=False)
               for g in range(HKV)]
        rg = [list(range(N_CORES))]
        gath = []

        # ---------------- Phase A: local K/V projection -------------------
        with (
            tc.tile_pool(name="kva", bufs=1) as kva,
            tc.tile_pool(name="kvw", bufs=2) as kvw,
            tc.tile_pool(name="kvps", bufs=2, space="PSUM") as kvps,
            tc.tile_pool(name="vps", bufs=2, space="PSUM") as vps,
        ):
            # PE warmup: the HAM clock gate holds the PE at 1.2 GHz until
            # ~3.4us of sustained matmul activity. The input DMA gates real
            # work for the first ~12us, so burn that window with dummy
            # matmuls on a memset scratch tile (no DMA dependencies) to
            # enter phase A's K/V projections at the full 2.4 GHz.
            warm = kva.tile([128, 512], BF)
            nc.gpsimd.memset(warm[:, 0:128], 0.0)
            for w in range(14):
                wps = vps.tile([128, 512], F32, tag="v")
                nc.tensor.matmul(wps[:], warm[:, 0:128], warm[:],
                                 start=True, stop=True)

            xkv_sb = kva.tile([128, HC, LKV], BF)
            wk_sb = kva.tile([128, HC, HKV * D], BF)
            wv_sb = kva.tile([128, HC, HKV * D], BF)
            for hc in range(HC):
                nc.sync.dma_start(
                    xkv_sb[:, hc, :], xkvT.ap()[hc * 128:(hc + 1) * 128, :])
                nc.sync.dma_start(
                    wk_sb[:, hc, :], wkT.ap()[hc * 128:(hc + 1) * 128, :])
                nc.sync.dma_start(
                    wv_sb[:, hc, :], wvT.ap()[hc * 128:(hc + 1) * 128, :])

            # resident loads behind the critical trio, still on the sync
            # queue: the gpsimd queue must stay clear for bounce writes and
            # collective triggers
            xq_sb = persist.tile([128, HC, QS], BF)
            nc.sync.dma_start(
                xq_sb[:], xqT.ap().rearrange("(a p) q -> p a q", p=128))
            mask_sb = persist.tile([128, 16, 512], BF)
            nc.sync.dma_start(
                mask_sb[:], maskD.ap().rearrange("(a p) q -> p a q", p=128))

            for g in range(HKV):
                gath.append(
                    dram.tile([N_CORES * 256 * LKV], BF, addr_space="Shared",
                              name=f"gath{g}", uniquify=False))

            def k_proj(g):
                ps = kvps.tile([128, LKV], F32, tag="kt")
                for hc in range(HC):
                    lhsT = wk_sb[:, hc, g * D:(g + 1) * D]
                    for nn in range(0, LKV, 512):
                        nc.tensor.matmul(
                            ps[:, nn:nn + 512], lhsT,
                            xkv_sb[:, hc, nn:nn + 512],
                            start=(hc == 0), stop=(hc == HC - 1))
                kt_loc = kvw.tile([128, LKV], BF, tag="ktloc")
                nc.vector.tensor_copy(kt_loc[:], ps[:])
                nc.gpsimd.dma_start(
                    bnc[g][0:128 * LKV].rearrange("(p c) -> p c", p=128),
                    kt_loc[:])

            # K0 first so AG0's trigger is gated only by the V loop
            k_proj(0)

            # V (N=512 across all 4 groups), scattering into the per-g
            # bounce tiles
            for lc in range(LKV // 128):
                ps = vps.tile([128, HKV * D], F32, tag="v")
                for hc in range(HC):
                    nc.tensor.matmul(
                        ps[:], xkv_sb[:, hc, lc * 128:(lc + 1) * 128],
                        wv_sb[:, hc, :],
                        start=(hc == 0), stop=(hc == HC - 1))
                v_loc = kvw.tile([128, HKV * D], BF, tag="vloc")
                nc.vector.tensor_copy(v_loc[:], ps[:])
                for g in range(HKV):
                    nc.gpsimd.dma_start(
                        bnc[g][128 * LKV + lc * 128 * D:
                               128 * LKV + (lc + 1) * 128 * D]
                        .rearrange("(p d) -> p d", p=128),
                        v_loc[:, g * D:(g + 1) * D])

            # AG2/AG3 are deferred into phase D so the gpsimd queue's
            # collective waits don't starve the partition reduces that run
            # between them
            nc.gpsimd.collective_compute(
                "AllGather", mybir.AluOpType.bypass, replica_groups=rg,
                ins=[bnc[0].opt()], outs=[gath[0].opt()])
            for g in range(1, HKV):
                k_proj(g)
                if g == 1:
                    nc.gpsimd.collective_compute(
                        "AllGather", mybir.AluOpType.bypass,
                        replica_groups=rg,
                        ins=[bnc[g].opt()], outs=[gath[g].opt()])

        # ---------------- Phase C: Q projection (overlaps AGs) ------------
        with (
            tc.tile_pool(name="qw", bufs=3) as qw,
            tc.tile_pool(name="qps", bufs=2, space="PSUM") as qps,
        ):
            for h in range(H):
                w_t = qw.tile([128, HC, 128], BF, tag="wq")
                nc.sync.dma_start(
                    w_t[:],
                    wqP.ap()[h * HID:(h + 1) * HID, :]
                    .rearrange("(a p) d -> p a d", p=128))
                ps = qps.tile([128, 512], F32, tag="q")
                for hc in range(HC):
                    nc.tensor.matmul(
                        ps[:, 0:QS], w_t[:, hc, :], xq_sb[:, hc, :],
                        start=(hc == 0), stop=(hc == HC - 1))
                for s in range(2):
                    nc.vector.tensor_copy(
                        qt_sb[:, s, h, :], ps[:, s * 128:(s + 1) * 128])

        # ---------------- Phase D: attention ------------------------------
        with (
            tc.tile_pool(name="kvstream", bufs=4) as kvstream,
            tc.tile_pool(name="attw", bufs=3) as attw,
            tc.tile_pool(name="accp", bufs=1) as accp,
            tc.tile_pool(name="wop", bufs=2) as wop,
            tc.tile_pool(name="stps", bufs=2, space="PSUM") as stps,
            tc.tile_pool(name="otps", bufs=1, space="PSUM") as otps,
            tc.tile_pool(name="fps", bufs=2, space="PSUM") as fps,
        ):
            dens = {}

            def norm(g):
                # normalize group g's attention output now that its
                # partition-reduced denominator has landed
                recip_f = attw.tile([128, 1024], F32, tag="recipf")
                # plain DVE reciprocal is iterative (~6.4 ns/elem -> 6.5us
                # here, stalling the PE at every group boundary); the 18-bit
                # Newton-Raphson approximation is ~5x faster and far more
                # accurate than the bf16 data anyway. den > 0 and finite.
                nc.vector.reciprocal_approx_fast(recip_f[:], dens[g][:])
                recip_b = attw.tile([128, 1024], BF, tag="recipb")
                nc.vector.tensor_copy(recip_b[:], recip_f[:])
                for s in range(2):
                    nc.vector.tensor_mul(
                        ao_sb[:, g, :, s * 128:(s + 1) * 128],
                        ao_sb[:, g, :, s * 128:(s + 1) * 128],
                        recip_b[:, s * 512:(s + 1) * 512]
                        .rearrange("p (h q) -> p h q", q=128))

            def o_proj(g):
                # fold group g into the output projection accumulator
                wo_g = wop.tile([128, 4, HID], BF, tag="wog")
                nc.sync.dma_start(
                    wo_g[:],
                    woT.ap()[g * 512:(g + 1) * 512, :]
                    .rearrange("(a p) d -> p a d", p=128))
                for jc in range(HC):
                    fp = fps.tile([128, 512], F32, tag="fp")
                    for hh in range(4):
                        nc.tensor.matmul(
                            fp[:, 0:QS],
                            wo_g[:, hh, jc * 128:(jc + 1) * 128],
                            ao_sb[:, g, hh, :],
                            start=(hh == 0), stop=(hh == 3))
                    if g == 0:
                        nc.vector.tensor_copy(out_acc[:, jc, :], fp[:, 0:QS])
                    else:
                        nc.vector.tensor_add(out_acc[:, jc, :],
                                             out_acc[:, jc, :], fp[:, 0:QS])

            for g in range(HKV):
                ot_ps = otps.tile([128, 1024], F32, tag="ot")
                acc2 = accp.tile([128, 2048], BF, tag="acc")   # even|odd kc
                accB = accp.tile([128, 512], BF, tag="accb")   # kc>=56, s1
                ex2 = None
                pend = None   # software pipeline: PV trails S/exp by one kc

                def emit_pv(p):
                    kc_p, vs_p, l_p, ex_s0, ex_s1 = p
                    if ex_s0 is not None:
                        nc.tensor.matmul(
                            ot_ps[:, 0:512], vs_p[:, l_p, :], ex_s0,
                            start=(kc_p == 0), stop=(kc_p == 55))
                    nc.tensor.matmul(
                        ot_ps[:, 512:1024], vs_p[:, l_p, :], ex_s1,
                        start=(kc_p == 0), stop=(kc_p == KC - 1))

                for r in range(N_CORES):
                    if g > 0 and r == 2:
                        # deferred norm of the previous group, one rank ahead
                        # of its O-proj so the DVE recip/mul chain finishes
                        # while the PE is still busy with this rank's matmuls
                        norm(g - 1)
                    if g > 0 and r == 3:
                        # the O-proj matmuls then interleave into the
                        # remaining ~5/8 of this group's attention
                        o_proj(g - 1)
                    base = r * 256 * LKV
                    kt_slab = kvstream.tile([128, LKV], BF, tag="kt")
                    nc.sync.dma_start(
                        kt_slab[:],
                        gath[g][base:base + 128 * LKV]
                        .rearrange("(p c) -> p c", p=128))
                    v_slab = kvstream.tile([128, LKV // 128, D], BF, tag="v")
                    # split across 4 DMA queues: one [128,8,128] transfer is
                    # 1024 256B descriptors serial on a single queue (~30us),
                    # which is borderline against the 10.5us/rank consume rate
                    for q4 in range(4):
                        nc.sync.dma_start(
                            v_slab[:, 2 * q4:2 * q4 + 2, :],
                            gath[g][base + 128 * LKV + q4 * 32768:
                                    base + 128 * LKV + (q4 + 1) * 32768]
                            .rearrange("(a p d) -> p a d", p=128, d=D))
                    for l in range(LKV // 128):
                        kc = r * (LKV // 128) + l
                        j = kc - BND
                        eps = kc & 1
                        ktc = kt_slab[:, l * 128:(l + 1) * 128]
                        st = stps.tile([128, 1024], F32, tag="st")
                        if kc < 56:
                            for s in range(2):
                                nc.tensor.matmul(
                                    st[:, s * 512:(s + 1) * 512], ktc,
                                    qt_sb[:, s, g * 4:(g + 1) * 4, :],
                                    start=True, stop=True)
                            if eps == 0:
                                ex2 = attw.tile([128, 2048], BF, tag="ex")
                            exh = ex2[:, eps * 1024:(eps + 1) * 1024]
                            nc.scalar.activation(
                                exh, st[:],
                                mybir.ActivationFunctionType.Exp, scale=SCALE)
                            if j >= 0:
                                # strip-0 boundary mask (ones rows harmless)
                                nc.vector.tensor_mul(
                                    ex2[:, eps * 1024:eps * 1024 + 512],
                                    ex2[:, eps * 1024:eps * 1024 + 512],
                                    mask_sb[:, j, :])
                            cur = (kc, v_slab, l,
                                   ex2[:, eps * 1024:eps * 1024 + 512],
                                   ex2[:, eps * 1024 + 512:(eps + 1) * 1024])
                            if pend is not None:
                                emit_pv(pend)
                            pend = cur
                            if eps == 1:
                                if kc == 1:
                                    nc.vector.tensor_copy(acc2[:], ex2[:])
                                else:
                                    nc.vector.tensor_add(
                                        acc2[:], acc2[:], ex2[:])
                        else:
                            # strip 0 fully masked for every core: s1 only
                            nc.tensor.matmul(
                                st[:, 512:1024], ktc,
                                qt_sb[:, 1, g * 4:(g + 1) * 4, :],
                                start=True, stop=True)
                            exB = attw.tile([128, 512], BF, tag="exb")
                            nc.scalar.activation(
                                exB[:], st[:, 512:1024],
                                mybir.ActivationFunctionType.Exp, scale=SCALE)
                            nc.vector.tensor_mul(
                                exB[:], exB[:], mask_sb[:, j, :])
                            cur = (kc, v_slab, l, None, exB[:])
                            if pend is not None:
                                emit_pv(pend)
                            pend = cur
                            if kc == 56:
                                nc.vector.tensor_copy(accB[:], exB[:])
                            else:
                                nc.vector.tensor_add(accB[:], accB[:], exB[:])
                emit_pv(pend)

                # copy attention output out unnormalized (frees ot_ps) and
                # kick off the denominator partition-reduce on gpsimd; the
                # normalization itself is deferred one group
                for s in range(2):
                    nc.vector.tensor_copy(
                        ao_sb[:, g, :, s * 128:(s + 1) * 128],
                        ot_ps[:, s * 512:(s + 1) * 512]
                        .rearrange("p (h q) -> p h q", q=128))
                accF = attw.tile([128, 1024], BF, tag="accf")
                nc.vector.tensor_add(accF[:], acc2[:, 0:1024],
                                     acc2[:, 1024:2048])
                nc.vector.tensor_add(accF[:, 512:1024], accF[:, 512:1024],
                                     accB[:])
                den = attw.tile([128, 1024], F32, tag="den")
                nc.gpsimd.partition_all_reduce(
                    den[:], accF[:], 128, bass_isa.ReduceOp.add)
                dens[g] = den
                if g < 2:
                    # AG2 after g0's reduce, AG3 after g1's: interleaving the
                    # gpsimd queue this way keeps each reduce from being
                    # starved by collective completion waits while still
                    # triggering each AG well before its group needs it
                    nc.gpsimd.collective_compute(
                        "AllGather", mybir.AluOpType.bypass,
                        replica_groups=rg,
                        ins=[bnc[g + 2].opt()], outs=[gath[g + 2].opt()])
            norm(HKV - 1)
            o_proj(HKV - 1)

        # ---------------- Phase F: store the accumulated output -----------
        for jc in range(HC):
            nc.sync.dma_start(outT.ap()[jc * 128:(jc + 1) * 128, :],
                              out_acc[:, jc, :])


def _get_nc():
    if "nc" not in _CACHE:
        _CACHE["nc"] = _build()
    return _CACHE["nc"]


def _make_in_maps(x_q, x_kv, Wq, Wk, Wv, Wo):
    xqT_full = np.ascontiguousarray(x_q[0].T)           # [HID, SL]
    xkvT_full = np.ascontiguousarray(x_kv[0].T)         # [HID, SKV]
    # head-major Wq: [16 heads][2048 hid][128 d]
    wqP = np.ascontiguousarray(
        Wq.T.reshape(HID, H, D).transpose(1, 0, 2)
    ).reshape(H * HID, D).astype(BF16)
    wkT = np.ascontiguousarray(Wk.T).astype(BF16)
    wvT = np.ascontiguousarray(Wv.T).astype(BF16)
    woT = np.ascontiguousarray(Wo.T).astype(BF16)

    in_maps = []
    kk = np.arange(128)
    for c in range(N_CORES):
        s0, s1 = c, 15 - c
        xqT = np.concatenate(
            [xqT_full[:, s0 * 128:(s0 + 1) * 128],
             xqT_full[:, s1 * 128:(s1 + 1) * 128]], axis=1).astype(BF16)
        xkvT = np.ascontiguousarray(
            xkvT_full[:, c * LKV:(c + 1) * LKV]).astype(BF16)
        # boundary masks: j<8 -> strip0 (=c), j>=8 -> strip1 (=15-c);
        # [128 keys, 128 q] tiled across the 4 heads of a group
        mask = np.zeros((16, 128, 128), dtype=np.float32)
        for jj in range(16):
            st_ = s0 if jj < 8 else s1
            key_g = (BND + jj) * 128 + kk
            q_g = RANK_OFF + st_ * 128 + kk
            mask[jj] = (key_g[:, None] <= q_g[None, :])
        mask4 = np.tile(mask, (1, 1, 4))                # [16, 128, 512]
        in_maps.append({
            "xqT": xqT, "xkvT": xkvT, "wqP": wqP, "wkT": wkT,
            "wvT": wvT, "woT": woT,
            "mask": mask4.reshape(16 * 128, 512).astype(BF16),
        })
    return in_maps


def _unshard(results):
    out = np.empty((1, SL, HID), dtype=np.float32)
    for c in range(N_CORES):
        outT = results[c]["outT"]                       # [HID, QS]
        s0, s1 = c, 15 - c
        out[0, s0 * 128:(s0 + 1) * 128, :] = outT[:, 0:128].T
        out[0, s1 * 128:(s1 + 1) * 128, :] = outT[:, 128:256].T
    return out


def kernel(x_q, x_kv, Wq, Wk, Wv, Wo, _trace=False, _result_box=None):
    nc = _get_nc()
    in_maps = _make_in_maps(x_q, x_kv, Wq, Wk, Wv, Wo)
    res = bass_utils.run_bass_kernel_spmd(
        nc, in_maps, core_ids=list(range(N_CORES)), trace=_trace)
    if _result_box is not None:
        _result_box.append(res)
    return _unshard(res.results)


# revision 28
# speedup vs baseline: 1.0294x; 1.0294x over previous
"""Ring-attention (context-parallel) kernel for 8 TRN2 NeuronCores.

Problem: x_q [1,2048,2048], x_kv [1,8192,2048], GQA attention (16 q heads,
4 kv heads, D=128) where q occupies global positions 6144..8191 of the
8192-long key sequence (causal on the last 2048 block, full attention on
the first 6144 keys), followed by an output projection.

Strategy (sequence/context parallel):
  - q rows split into 16 strips of 128; core c owns strips {c, 15-c}
    (folded pairing balances the causal wedge).
  - x_kv sequence-sharded 8 x 1024 rows; each core projects its local
    K/V shard, then one AllGather PER HEAD GROUP shares the full K^T / V.
    The V projection runs first and each AG launches right after its
    group's K^T finishes, so AG0 completes long before attention needs it.
  - Attention q-columns are ordered (strip, head, q128) so every S matmul
    is a single N=512 pass per strip sharing one K-chunk weight load, and
    fully-causal-masked strip halves are skipped (kc>=56 computes only the
    high strip).
  - Softmax denominator: bf16 chunk accumulation on DVE, then one
    gpsimd.partition_all_reduce (sum over partitions, broadcast) per head
    group -- no TensorE ones-matmuls, no extra PSUM banks.
  - Output projection for group g is emitted after group g+1's attention
    so the normalization chain never stalls the in-order PE queue.
"""

import numpy as np
import ml_dtypes

import concourse.bass as bass
import concourse.mybir as mybir
import concourse.tile as tile
from concourse import bacc, bass_isa, bass_utils

BF16 = ml_dtypes.bfloat16
F32 = mybir.dt.float32
BF = mybir.dt.bfloat16

N_CORES = 8
H = 16          # query heads
HKV = 4         # kv heads
D = 128         # head dim
HID = H * D     # 2048
SL = 2048       # q rows (global)
SKV = 8192      # kv rows (global)
QS = 256        # q rows per core (2 strips of 128)
LKV = SKV // N_CORES   # 1024 local kv rows
HC = HID // 128        # 16 hid chunks
KC = SKV // 128        # 64 key chunks
RANK_OFF = SKV - SL    # 6144: global position of q row 0
BND = RANK_OFF // 128  # 48: first key chunk needing a causal mask
SCALE = 1.0 / float(np.sqrt(D))

_CACHE = {}


def _build():
    nc = bacc.Bacc("TRN2", target_bir_lowering=False, debug=False,
                   num_devices=N_CORES)

    xqT = nc.dram_tensor("xqT", [HID, QS], BF, kind="ExternalInput")
    xkvT = nc.dram_tensor("xkvT", [HID, LKV], BF, kind="ExternalInput")
    wqP = nc.dram_tensor("wqP", [H * HID, D], BF, kind="ExternalInput")
    wkT = nc.dram_tensor("wkT", [HID, HKV * D], BF, kind="ExternalInput")
    wvT = nc.dram_tensor("wvT", [HID, HKV * D], BF, kind="ExternalInput")
    woT = nc.dram_tensor("woT", [HID, HID], BF, kind="ExternalInput")
    # boundary causal masks, (strip,head,q) col order: j<8 -> strip0 mask,
    # j>=8 -> strip1 mask, each [128 keys, 4 heads x 128 q]
    maskD = nc.dram_tensor("mask", [16 * 128, 512], BF, kind="ExternalInput")
    outT = nc.dram_tensor("outT", [HID, QS], F32, kind="ExternalOutput")

    with tile.TileContext(nc) as tc:
        _body(nc, tc, xqT, xkvT, wqP, wkT, wvT, woT, maskD, outT)
    nc.compile()
    return nc


def _body(nc, tc, xqT, xkvT, wqP, wkT, wvT, woT, maskD, outT):
    from contextlib import ExitStack
    ctx = ExitStack()
    with ctx:
        persist = ctx.enter_context(tc.tile_pool(name="persist", bufs=1))
        dram = ctx.enter_context(tc.tile_pool(name="dram", bufs=1, space="DRAM"))

        # resident tiles
        qt_sb = persist.tile([128, 2, H, 128], BF)    # Q^T [D, strip, head, q]
        ao_sb = persist.tile([128, HKV, 4, QS], BF)   # normalized O^T per g
        out_acc = persist.tile([128, HC, QS], F32)

        # per-g AllGather bounce: bytes [0:128*LKV) = K^T_g [D, LKV],
        # [128*LKV:256*LKV) = V_g [LKV, D] row-major
        bnc = [dram.tile([256 * LKV], BF, name=f"bnc{g}", uniquify=False)
               for g in range(HKV)]
        rg = [list(range(N_CORES))]
        gath = []

        # ---------------- Phase A: local K/V projection -------------------
        with (
            tc.tile_pool(name="kva", bufs=1) as kva,
            tc.tile_pool(name="kvw", bufs=2) as kvw,
            tc.tile_pool(name="kvps", bufs=2, space="PSUM") as kvps,
            tc.tile_pool(name="vps", bufs=2, space="PSUM") as vps,
        ):
            xkv_sb = kva.tile([128, HC, LKV], BF)
            wk_sb = kva.tile([128, HC, HKV * D], BF)
            wv_sb = kva.tile([128, HC, HKV * D], BF)
            for hc in range(HC):
                nc.sync.dma_start(
                    xkv_sb[:, hc, :], xkvT.ap()[hc * 128:(hc + 1) * 128, :])
                nc.sync.dma_start(
                    wk_sb[:, hc, :], wkT.ap()[hc * 128:(hc + 1) * 128, :])
                nc.sync.dma_start(
                    wv_sb[:, hc, :], wvT.ap()[hc * 128:(hc + 1) * 128, :])

            # resident loads behind the critical trio, still on the sync
            # queue: the gpsimd queue must stay clear for bounce writes and
            # collective triggers
            xq_sb = persist.tile([128, HC, QS], BF)
            nc.sync.dma_start(
                xq_sb[:], xqT.ap().rearrange("(a p) q -> p a q", p=128))
            mask_sb = persist.tile([128, 16, 512], BF)
            nc.sync.dma_start(
                mask_sb[:], maskD.ap().rearrange("(a p) q -> p a q", p=128))

            for g in range(HKV):
                gath.append(
                    dram.tile([N_CORES * 256 * LKV], BF, addr_space="Shared",
                              name=f"gath{g}", uniquify=False))

            def k_proj(g):
                ps = kvps.tile([128, LKV], F32, tag="kt")
                for hc in range(HC):
                    lhsT = wk_sb[:, hc, g * D:(g + 1) * D]
                    for nn in range(0, LKV, 512):
                        nc.tensor.matmul(
                            ps[:, nn:nn + 512], lhsT,
                            xkv_sb[:, hc, nn:nn + 512],
                            start=(hc == 0), stop=(hc == HC - 1))
                kt_loc = kvw.tile([128, LKV], BF, tag="ktloc")
                nc.vector.tensor_copy(kt_loc[:], ps[:])
                nc.gpsimd.dma_start(
                    bnc[g][0:128 * LKV].rearrange("(p c) -> p c", p=128),
                    kt_loc[:])

            # K0 first so AG0's trigger is gated only by the V loop
            k_proj(0)

            # V (N=512 across all 4 groups), scattering into the per-g
            # bounce tiles
            for lc in range(LKV // 128):
                ps = vps.tile([128, HKV * D], F32, tag="v")
                for hc in range(HC):
                    nc.tensor.matmul(
                        ps[:], xkv_sb[:, hc, lc * 128:(lc + 1) * 128],
                        wv_sb[:, hc, :],
                        start=(hc == 0), stop=(hc == HC - 1))
                v_loc = kvw.tile([128, HKV * D], BF, tag="vloc")
                nc.vector.tensor_copy(v_loc[:], ps[:])
                for g in range(HKV):
                    nc.gpsimd.dma_start(
                        bnc[g][128 * LKV + lc * 128 * D:
                               128 * LKV + (lc + 1) * 128 * D]
                        .rearrange("(p d) -> p d", p=128),
                        v_loc[:, g * D:(g + 1) * D])

            # AG2/AG3 are deferred into phase D so the gpsimd queue's
            # collective waits don't starve the partition reduces that run
            # between them
            nc.gpsimd.collective_compute(
                "AllGather", mybir.AluOpType.bypass, replica_groups=rg,
                ins=[bnc[0].opt()], outs=[gath[0].opt()])
            for g in range(1, HKV):
                k_proj(g)
                if g == 1:
                    nc.gpsimd.collective_compute(
                        "AllGather", mybir.AluOpType.bypass,
                        replica_groups=rg,
                        ins=[bnc[g].opt()], outs=[gath[g].opt()])

        # ---------------- Phase C: Q projection (overlaps AGs) ------------
        with (
            tc.tile_pool(name="qw", bufs=3) as qw,
            tc.tile_pool(name="qps", bufs=2, space="PSUM") as qps,
        ):
            for h in range(H):
                w_t = qw.tile([128, HC, 128], BF, tag="wq")
                nc.sync.dma_start(
                    w_t[:],
                    wqP.ap()[h * HID:(h + 1) * HID, :]
                    .rearrange("(a p) d -> p a d", p=128))
                ps = qps.tile([128, 512], F32, tag="q")
                for hc in range(HC):
                    nc.tensor.matmul(
                        ps[:, 0:QS], w_t[:, hc, :], xq_sb[:, hc, :],
                        start=(hc == 0), stop=(hc == HC - 1))
                for s in range(2):
                    nc.vector.tensor_copy(
                        qt_sb[:, s, h, :], ps[:, s * 128:(s + 1) * 128])

        # ---------------- Phase D: attention ------------------------------
        with (
            tc.tile_pool(name="kvstream", bufs=6) as kvstream,
            tc.tile_pool(name="attw", bufs=3) as attw,
            tc.tile_pool(name="accp", bufs=1) as accp,
            tc.tile_pool(name="wop", bufs=2) as wop,
            tc.tile_pool(name="stps", bufs=2, space="PSUM") as stps,
            tc.tile_pool(name="otps", bufs=1, space="PSUM") as otps,
            tc.tile_pool(name="fps", bufs=2, space="PSUM") as fps,
        ):
            dens = {}

            def norm(g):
                # normalize group g's attention output now that its
                # partition-reduced denominator has landed
                recip_f = attw.tile([128, 1024], F32, tag="recipf")
                # plain DVE reciprocal is iterative (~6.4 ns/elem -> 6.5us
                # here, stalling the PE at every group boundary); the 18-bit
                # Newton-Raphson approximation is ~5x faster and far more
                # accurate than the bf16 data anyway. den > 0 and finite.
                nc.vector.reciprocal_approx_fast(recip_f[:], dens[g][:])
                recip_b = attw.tile([128, 1024], BF, tag="recipb")
                nc.vector.tensor_copy(recip_b[:], recip_f[:])
                for s in range(2):
                    nc.vector.tensor_mul(
                        ao_sb[:, g, :, s * 128:(s + 1) * 128],
                        ao_sb[:, g, :, s * 128:(s + 1) * 128],
                        recip_b[:, s * 512:(s + 1) * 512]
                        .rearrange("p (h q) -> p h q", q=128))

            def o_proj(g):
                # fold group g into the output projection accumulator
                wo_g = wop.tile([128, 4, HID], BF, tag="wog")
                nc.sync.dma_start(
                    wo_g[:],
                    woT.ap()[g * 512:(g + 1) * 512, :]
                    .rearrange("(a p) d -> p a d", p=128))
                for jc in range(HC):
                    fp = fps.tile([128, 512], F32, tag="fp")
                    for hh in range(4):
                        nc.tensor.matmul(
                            fp[:, 0:QS],
                            wo_g[:, hh, jc * 128:(jc + 1) * 128],
                            ao_sb[:, g, hh, :],
                            start=(hh == 0), stop=(hh == 3))
                    if g == 0:
                        nc.vector.tensor_copy(out_acc[:, jc, :], fp[:, 0:QS])
                    else:
                        nc.vector.tensor_add(out_acc[:, jc, :],
                                             out_acc[:, jc, :], fp[:, 0:QS])

            for g in range(HKV):
                ot_ps = otps.tile([128, 1024], F32, tag="ot")
                acc2 = accp.tile([128, 2048], BF, tag="acc")   # even|odd kc
                accB = accp.tile([128, 512], BF, tag="accb")   # kc>=56, s1
                ex2 = None
                pend = None   # software pipeline: PV trails S/exp by one kc

                def emit_pv(p):
                    kc_p, vs_p, l_p, ex_s0, ex_s1 = p
                    if ex_s0 is not None:
                        nc.tensor.matmul(
                            ot_ps[:, 0:512], vs_p[:, l_p, :], ex_s0,
                            start=(kc_p == 0), stop=(kc_p == 55))
                    nc.tensor.matmul(
                        ot_ps[:, 512:1024], vs_p[:, l_p, :], ex_s1,
                        start=(kc_p == 0), stop=(kc_p == KC - 1))

                # g0's partition-reduce lands late (the gpsimd queue drains
                # all collective waits first), so norm(0) waits two extra
                # ranks; later groups' reduces are ready sooner
                nr = 4 if g == 1 else 2
                for r in range(N_CORES):
                    if g > 0 and r == nr:
                        # deferred norm of the previous group, one rank ahead
                        # of its O-proj so the DVE recip/mul chain finishes
                        # while the PE is still busy with this rank's matmuls
                        norm(g - 1)
                    if g > 0 and r == nr + 1:
                        # the O-proj matmuls then interleave into the
                        # remaining ranks of this group's attention
                        o_proj(g - 1)
                    base = r * 256 * LKV
                    kt_slab = kvstream.tile([128, LKV], BF, tag="kt")
                    nc.sync.dma_start(
                        kt_slab[:],
                        gath[g][base:base + 128 * LKV]
                        .rearrange("(p c) -> p c", p=128))
                    v_slab = kvstream.tile([128, LKV // 128, D], BF, tag="v")
                    # split across 4 DMA queues: one [128,8,128] transfer is
                    # 1024 256B descriptors serial on a single queue (~30us),
                    # which is borderline against the 10.5us/rank consume rate
                    for q4 in range(4):
                        nc.sync.dma_start(
                            v_slab[:, 2 * q4:2 * q4 + 2, :],
                            gath[g][base + 128 * LKV + q4 * 32768:
                                    base + 128 * LKV + (q4 + 1) * 32768]
                            .rearrange("(a p d) -> p a d", p=128, d=D))
                    for l in range(LKV // 128):
                        kc = r * (LKV // 128) + l
                        j = kc - BND
                        eps = kc & 1
                        ktc = kt_slab[:, l * 128:(l + 1) * 128]
                        st = stps.tile([128, 1024], F32, tag="st")
                        if kc < 56:
                            for s in range(2):
                                nc.tensor.matmul(
                                    st[:, s * 512:(s + 1) * 512], ktc,
                                    qt_sb[:, s, g * 4:(g + 1) * 4, :],
                                    start=True, stop=True)
                            if eps == 0:
                                ex2 = attw.tile([128, 2048], BF, tag="ex")
                            exh = ex2[:, eps * 1024:(eps + 1) * 1024]
                            nc.scalar.activation(
                                exh, st[:],
                                mybir.ActivationFunctionType.Exp, scale=SCALE)
                            if j >= 0:
                                # strip-0 boundary mask (ones rows harmless)
                                nc.vector.tensor_mul(
                                    ex2[:, eps * 1024:eps * 1024 + 512],
                                    ex2[:, eps * 1024:eps * 1024 + 512],
                                    mask_sb[:, j, :])
                            cur = (kc, v_slab, l,
                                   ex2[:, eps * 1024:eps * 1024 + 512],
                                   ex2[:, eps * 1024 + 512:(eps + 1) * 1024])
                            if pend is not None:
                                emit_pv(pend)
                            pend = cur
                            if eps == 1:
                                if kc == 1:
                                    nc.vector.tensor_copy(acc2[:], ex2[:])
                                else:
                                    nc.vector.tensor_add(
                                        acc2[:], acc2[:], ex2[:])
                        else:
                            # strip 0 fully masked for every core: s1 only
                            nc.tensor.matmul(
                                st[:, 512:1024], ktc,
                                qt_sb[:, 1, g * 4:(g + 1) * 4, :],
                                start=True, stop=True)
                            exB = attw.tile([128, 512], BF, tag="exb")
                            nc.scalar.activation(
                                exB[:], st[:, 512:1024],
                                mybir.ActivationFunctionType.Exp, scale=SCALE)
                            nc.vector.tensor_mul(
                                exB[:], exB[:], mask_sb[:, j, :])
                            cur = (kc, v_slab, l, None, exB[:])
                            if pend is not None:
                                emit_pv(pend)
                            pend = cur
                            if kc == 56:
                                nc.vector.tensor_copy(accB[:], exB[:])
                            else:
                                nc.vector.tensor_add(accB[:], accB[:], exB[:])
                emit_pv(pend)

                # copy attention output out unnormalized (frees ot_ps) and
                # kick off the denominator partition-reduce on gpsimd; the
                # normalization itself is deferred one group
                for s in range(2):
                    nc.vector.tensor_copy(
                        ao_sb[:, g, :, s * 128:(s + 1) * 128],
                        ot_ps[:, s * 512:(s + 1) * 512]
                        .rearrange("p (h q) -> p h q", q=128))
                accF = attw.tile([128, 1024], BF, tag="accf")
                nc.vector.tensor_add(accF[:], acc2[:, 0:1024],
                                     acc2[:, 1024:2048])
                nc.vector.tensor_add(accF[:, 512:1024], accF[:, 512:1024],
                                     accB[:])
                den = attw.tile([128, 1024], F32, tag="den")
                nc.gpsimd.partition_all_reduce(
                    den[:], accF[:], 128, bass_isa.ReduceOp.add)
                dens[g] = den
                if g < 2:
                    # AG2 after g0's reduce, AG3 after g1's: interleaving the
                    # gpsimd queue this way keeps each reduce from being
                    # starved by collective completion waits while still
                    # triggering each AG well before its group needs it
                    nc.gpsimd.collective_compute(
                        "AllGather", mybir.AluOpType.bypass,
                        replica_groups=rg,
                        ins=[bnc[g + 2].opt()], outs=[gath[g + 2].opt()])
            norm(HKV - 1)
            o_proj(HKV - 1)

        # ---------------- Phase F: store the accumulated output -----------
        for jc in range(HC):
            nc.sync.dma_start(outT.ap()[jc * 128:(jc + 1) * 128, :],
                              out_acc[:, jc, :])


def _get_nc():
    if "nc" not in _CACHE:
        _CACHE["nc"] = _build()
    return _CACHE["nc"]


def _make_in_maps(x_q, x_kv, Wq, Wk, Wv, Wo):
    xqT_full = np.ascontiguousarray(x_q[0].T)           # [HID, SL]
    xkvT_full = np.ascontiguousarray(x_kv[0].T)         # [HID, SKV]
    # head-major Wq: [16 heads][2048 hid][128 d]
    wqP = np.ascontiguousarray(
        Wq.T.reshape(HID, H, D).transpose(1, 0, 2)
    ).reshape(H * HID, D).astype(BF16)
    wkT = np.ascontiguousarray(Wk.T).astype(BF16)
    wvT = np.ascontiguousarray(Wv.T).astype(BF16)
    woT = np.ascontiguousarray(Wo.T).astype(BF16)

    in_maps = []
    kk = np.arange(128)
    for c in range(N_CORES):
        s0, s1 = c, 15 - c
        xqT = np.concatenate(
            [xqT_full[:, s0 * 128:(s0 + 1) * 128],
             xqT_full[:, s1 * 128:(s1 + 1) * 128]], axis=1).astype(BF16)
        xkvT = np.ascontiguousarray(
            xkvT_full[:, c * LKV:(c + 1) * LKV]).astype(BF16)
        # boundary masks: j<8 -> strip0 (=c), j>=8 -> strip1 (=15-c);
        # [128 keys, 128 q] tiled across the 4 heads of a group
        mask = np.zeros((16, 128, 128), dtype=np.float32)
        for jj in range(16):
            st_ = s0 if jj < 8 else s1
            key_g = (BND + jj) * 128 + kk
            q_g = RANK_OFF + st_ * 128 + kk
            mask[jj] = (key_g[:, None] <= q_g[None, :])
        mask4 = np.tile(mask, (1, 1, 4))                # [16, 128, 512]
        in_maps.append({
            "xqT": xqT, "xkvT": xkvT, "wqP": wqP, "wkT": wkT,
            "wvT": wvT, "woT": woT,
            "mask": mask4.reshape(16 * 128, 512).astype(BF16),
        })
    return in_maps


def _unshard(results):
    out = np.empty((1, SL, HID), dtype=np.float32)
    for c in range(N_CORES):
        outT = results[c]["outT"]                       # [HID, QS]
        s0, s1 = c, 15 - c
        out[0, s0 * 128:(s0 + 1) * 128, :] = outT[:, 0:128].T
        out[0, s1 * 128:(s1 + 1) * 128, :] = outT[:, 128:256].T
    return out


def kernel(x_q, x_kv, Wq, Wk, Wv, Wo, _trace=False, _result_box=None):
    nc = _get_nc()
    in_maps = _make_in_maps(x_q, x_kv, Wq, Wk, Wv, Wo)
    res = bass_utils.run_bass_kernel_spmd(
        nc, in_maps, core_ids=list(range(N_CORES)), trace=_trace)
    if _result_box is not None:
        _result_box.append(res)
    return _unshard(res.results)
